# revision 1
# baseline (speedup 1.0000x reference)
"""Trainium2 Bass kernel for the BINN convnet problem.

Computation (per row b of inp, all column indices mod D=128):
    x[b, j]  = (c1[j] * a[b, j+1] - c2[j] * a[b, j-2]) * a[b, j-1]
    out      = x + a @ W_lin.T + b_lin
with c1[j] = w[j,0]*w[j,2], c2[j] = w[j,1]*w[j,2], except j==1 where the
outer factor is w[1,0] instead of w[1,2].

Strategy: pure data parallel across 8 NeuronCores (batch split).  On each
core, per 512-row compute subtile (1024-row DMA tiles, (p q) layout so each
partition line is one contiguous 4 KB DRAM chunk):

  1. PE-transposes A -> A^T per 128-row block (plain fp32 transpose mode);
     ScalarE evacuates PSUM->SBUF, rounding to float32r.
  2. The stencil's linear part g[b,j] = c1[j] a[b,j+1] - c2[j] a[b,j-2] is
     a constant banded matrix G.  One float32r matmul per block with
     lhsT = A^T-block (stationary) and rhs = [G^T | W_lin^T] (moving,
     N=256 -> full PE rate) produces g and mm = a @ W_lin.T both in
     NATURAL layout in PSUM.  No transpose-back is needed.
  3. DVE: x = a[:, j-1] * g with the j-1 roll expressed as shifted
     free-dim access patterns on the natural A tile (main + 1-col wrap),
     then out = x + mm written straight to SBUF.
  4. GpSimd adds the column bias b_lin (broadcast constant) in SBUF.
  5. Store the natural tile.
"""

import os
import sys

import numpy as np

if os.path.isdir("/opt/trn_rl_repo") and "/opt/trn_rl_repo" not in sys.path:
    sys.path.insert(0, "/opt/trn_rl_repo")

import concourse.mybir as mybir
import concourse.tile as tile
from concourse import bacc
from concourse.bass_utils import run_bass_kernel_spmd

D = 128          # feature dim
N_CORES = 8
SUB = 512        # rows per compute subtile
DMA_ROWS = 2048  # rows per DMA tile
F32 = mybir.dt.float32
F32R = mybir.dt.float32r
BIAS_ON_POOL = True


def build_program(nrows: int):
    """Build the single-core Bass program (SPMD across cores)."""
    assert nrows % DMA_ROWS == 0
    ndma = nrows // DMA_ROWS
    nsub = DMA_ROWS // SUB  # compute subtiles per DMA tile (2)
    QB = SUB // D           # 128-row blocks per compute subtile (4)

    nc = bacc.Bacc("TRN2", debug=False, target_bir_lowering=False)

    inp = nc.declare_dram_parameter("inp", [nrows, D], F32, isOutput=False)
    gw = nc.declare_dram_parameter("gw", [D, 2 * D], F32, isOutput=False)
    bbc = nc.declare_dram_parameter("bbc", [D, DMA_ROWS], F32, isOutput=False)
    bmask = nc.declare_dram_parameter("bmask", [1, SUB], F32, isOutput=False)
    ones = nc.declare_dram_parameter("ones", [1, D], F32, isOutput=False)
    ident = nc.declare_dram_parameter("ident", [D, D], F32, isOutput=False)
    out = nc.declare_dram_parameter("out", [nrows, D], F32, isOutput=True)

    with tile.TileContext(nc) as tc:
        with (
            tc.tile_pool(name="const", bufs=1) as const_pool,
            tc.tile_pool(name="a_sb", bufs=6) as a_pool,
            tc.tile_pool(name="at_sb", bufs=4) as at_pool,
            tc.tile_pool(name="xt_sb", bufs=4) as xt_pool,
            tc.tile_pool(name="o_sb", bufs=6) as o_pool,
            tc.tile_pool(name="at_ps", bufs=2, space="PSUM") as atps_pool,
            tc.tile_pool(name="gw_ps", bufs=3, space="PSUM") as gwps_pool,
        ):
            # --- constants, loaded once ---
            gw_sb = const_pool.tile([D, 2 * D], F32)
            bbc_sb = const_pool.tile([D, DMA_ROWS], F32)
            bmask_sb = const_pool.tile([1, SUB], F32)
            ones_sb = const_pool.tile([1, D], F32)
            id_sb = const_pool.tile([D, D], F32)
            nc.sync.dma_start(out=gw_sb[:], in_=gw[:, :])
            nc.sync.dma_start(out=bbc_sb[:], in_=bbc[:, :])
            nc.sync.dma_start(out=bmask_sb[:], in_=bmask[:, :])
            nc.sync.dma_start(out=ones_sb[:], in_=ones[:, :])
            nc.sync.dma_start(out=id_sb[:], in_=ident[:, :])

            # fp32r matmul operands must be produced by an fp32r-rounding
            # instruction (walrus checkMatmultFP32r) — round the constants once.
            gw_rt = const_pool.tile([D, 2 * D], F32R)
            bmask_rt = const_pool.tile([1, SUB], F32R)
            ones_rt = const_pool.tile([1, D], F32R)
            id_rt = const_pool.tile([D, D], F32R)
            nc.vector.tensor_copy(out=gw_rt[:], in_=gw_sb[:])
            nc.vector.tensor_copy(out=bmask_rt[:], in_=bmask_sb[:])
            nc.vector.tensor_copy(out=ones_rt[:], in_=ones_sb[:])
            nc.vector.tensor_copy(out=id_rt[:], in_=id_sb[:])

            # Software pipeline by one subtile: PE's stream per step is
            # [trA(k), GW(k-1)] so PE transposes subtile k while ScalarE
            # evacuates A^T of k-1 — no PE stall on the evac round-trip.
            nsubs = ndma * nsub
            st = {}  # k -> (td, f0, a_sb, o_sb, at_ps, at_sb)

            def emit_front(k):
                td, ts = divmod(k, nsub)
                if ts == 0:
                    r0 = td * DMA_ROWS
                    # (p q) layout: partition p holds DMA_ROWS/128 consecutive
                    # DRAM rows -> one contiguous DRAM chunk per partition.
                    a_sb = a_pool.tile([D, DMA_ROWS], F32, tag="a")
                    src = inp[r0 : r0 + DMA_ROWS, :].rearrange(
                        "(p q) d -> p q d", p=D
                    )
                    nc.sync.dma_start(
                        out=a_sb[:].rearrange("p (q d) -> p q d", d=D), in_=src
                    )
                    o_sb = o_pool.tile([D, DMA_ROWS], F32, tag="o")
                else:
                    _, _, a_sb, o_sb, _, _ = st[k - 1]
                f0 = ts * SUB

                # --- PE transpose A -> A^T (per 128 block, plain fp32) ---
                at_ps = atps_pool.tile([D, SUB], F32, tag="atps")
                for q in range(QB):
                    nc.tensor.matmul(
                        out=at_ps[:, q * D : (q + 1) * D],
                        lhsT=a_sb[:, f0 + q * D : f0 + (q + 1) * D],
                        rhs=id_sb[:],
                        is_transpose=True,
                        start=True,
                        stop=True,
                    )
                st[k] = (td, f0, a_sb, o_sb, at_ps, None)

            def emit_evac(k):
                td, f0, a_sb, o_sb, at_ps, _ = st[k]
                # evacuate A^T to SBUF (ScalarE), rounding to fp32r
                at_sb = at_pool.tile([D, SUB], F32R, tag="at")
                nc.scalar.copy(out=at_sb[:], in_=at_ps[:])
                st[k] = (td, f0, a_sb, o_sb, at_sb, None)

            def emit_gw(k):
                td, f0, a_sb, o_sb, at_sb, _ = st[k]
                # --- [g | mm] per block, natural layout, in PSUM ---
                # gw_ps free layout: [g0|m0|g1|m1|g2|m2|g3|m3], 2 banks
                gw_ps = gwps_pool.tile([D, 4 * 2 * D], F32, tag="gwps")
                for q in range(QB):
                    nc.tensor.matmul(
                        out=gw_ps[:, q * 2 * D : (q + 1) * 2 * D],
                        lhsT=at_sb[:, q * D : (q + 1) * D],
                        rhs=gw_rt[:],
                        start=True,
                        stop=BIAS_ON_POOL,
                    )
                if not BIAS_ON_POOL:
                    # accumulate b_lin onto the mm halves (masked rhs)
                    for h in range(2):
                        nc.tensor.matmul(
                            out=gw_ps[:, h * SUB : (h + 1) * SUB],
                            lhsT=ones_rt[:],
                            rhs=bmask_rt[:],
                            start=False,
                            stop=True,
                        )
                st[k] = (td, f0, a_sb, o_sb, at_sb, gw_ps)

            def emit_back(k):
                td, f0, a_sb, o_sb, _, gw_ps = st[k]
                gw3 = gw_ps[:].rearrange("p (q c) -> p q c", c=2 * D)
                a3 = a_sb[:, f0 : f0 + SUB].rearrange("p (q d) -> p q d", d=D)
                o3 = o_sb[:, f0 : f0 + SUB].rearrange("p (q d) -> p q d", d=D)

                # --- x = a[:, j-1] * g (DVE; shifted free-dim APs) ---
                xt_sb = xt_pool.tile([D, SUB], F32, tag="xt")
                x3 = xt_sb[:].rearrange("p (q d) -> p q d", d=D)
                nc.vector.tensor_mul(
                    out=x3[:, :, 1:D], in0=a3[:, :, 0 : D - 1], in1=gw3[:, :, 1:D]
                )
                nc.vector.tensor_mul(
                    out=x3[:, :, 0:1], in0=a3[:, :, D - 1 : D], in1=gw3[:, :, 0:1]
                )

                # --- out = x + mm (DVE, straight to SBUF) ---
                nc.vector.tensor_add(
                    out=o3[:, :, :], in0=xt_sb[:], in1=gw3[:, :, D : 2 * D]
                )

                if BIAS_ON_POOL:
                    # --- += b_lin broadcast (GpSimd, SBUF only) ---
                    nc.gpsimd.tensor_tensor(
                        out=o_sb[:, f0 : f0 + SUB],
                        in0=o_sb[:, f0 : f0 + SUB],
                        in1=bbc_sb[:, 0:SUB],
                        op=mybir.AluOpType.add,
                    )

            def emit_store(k):
                td, _, _, o_sb, _, _ = st[k]
                if k % nsub == nsub - 1:
                    # --- store (Scalar HWDGE ring; loads use the SP ring).
                    # Deferred one extra stage so the store's semaphore wait
                    # (on the GpSimd bias) never stalls ACT's queue ahead of
                    # the next evacuations. ---
                    r0 = td * DMA_ROWS
                    dst = out[r0 : r0 + DMA_ROWS, :].rearrange(
                        "(p q) d -> p q d", p=D
                    )
                    nc.scalar.dma_start(
                        out=dst, in_=o_sb[:].rearrange("p (q d) -> p q d", d=D)
                    )

            # 5-stage pipeline:
            # [trA(k)] [evac(k-1)] [GW(k-2)] [TT+bias(k-3)] [store(k-4)]
            for step in range(nsubs + 4):
                if step < nsubs:
                    emit_front(step)
                if step >= 1 and step - 1 < nsubs:
                    emit_evac(step - 1)
                if step >= 2 and step - 2 < nsubs:
                    emit_gw(step - 2)
                if step >= 3 and step - 3 < nsubs:
                    emit_back(step - 3)
                if step >= 4 and step - 4 < nsubs:
                    emit_store(step - 4)

    nc.compile()
    return nc


def make_consts(w: np.ndarray, W_lin: np.ndarray, b_lin: np.ndarray):
    """Host-side constant preparation (all tiny)."""
    w = np.asarray(w, np.float64)
    c1 = w[:, 0] * w[:, 2]
    c2 = w[:, 1] * w[:, 2]
    # column 1 uses w[1,0] as the outer factor (faithful to source)
    c1[1] = w[1, 0] * w[1, 0]
    c2[1] = w[1, 1] * w[1, 0]

    j = np.arange(D)
    G = np.zeros((D, D), np.float64)
    G[j, (j + 1) % D] += c1
    G[j, (j - 2) % D] -= c2

    gwm = np.zeros((D, 2 * D), np.float32)
    gwm[:, :D] = G.T           # gw[d, j] = G[j, d]
    gwm[:, D:] = np.asarray(W_lin, np.float64).T  # gw[d, D+j] = W_lin[j, d]

    b32 = np.asarray(b_lin, np.float32)
    bbc = np.ascontiguousarray(np.tile(b32, (D, DMA_ROWS // D)))  # [128, 1024]
    bmask = np.zeros((1, SUB), np.float32)
    bmask[0, D : 2 * D] = b32
    bmask[0, 3 * D : 4 * D] = b32
    ones = np.ones((1, D), np.float32)
    ident = np.eye(D, dtype=np.float32)
    return {"gw": gwm, "bbc": bbc, "bmask": bmask, "ones": ones, "ident": ident}


_PROGRAM_CACHE: dict[int, object] = {}
TRACE = False      # test-only: capture NTFF profile on the next kernel() call
TRACE_DIR = None   # test-only: where to keep NTFF/perfetto artifacts
LAST_RESULT = None  # test-only: BassKernelResults of the last run


def _get_program(nrows: int):
    if nrows not in _PROGRAM_CACHE:
        _PROGRAM_CACHE[nrows] = build_program(nrows)
    return _PROGRAM_CACHE[nrows]


def kernel(**inputs) -> np.ndarray:
    inp = np.ascontiguousarray(np.asarray(inputs["inp"], np.float32))
    w = np.asarray(inputs["w"], np.float32)
    W_lin = np.asarray(inputs["W_lin"], np.float32)
    b_lin = np.asarray(inputs["b_lin"], np.float32)

    B = inp.shape[0]
    assert inp.shape[1] == D and B % N_CORES == 0
    nrows = B // N_CORES

    consts = make_consts(w, W_lin, b_lin)
    shards = inp.reshape(N_CORES, nrows, D)

    nc = _get_program(nrows)
    in_maps = [{"inp": shards[i], **consts} for i in range(N_CORES)]
    res = run_bass_kernel_spmd(
        nc, in_maps, list(range(N_CORES)), trace=TRACE, tmpdir=TRACE_DIR
    )
    global LAST_RESULT
    LAST_RESULT = res
    return np.concatenate([res.results[i]["out"] for i in range(N_CORES)], axis=0)


if __name__ == "__main__":
    # quick smoke test on random data vs numpy
    rng = np.random.default_rng(0)
    B = N_CORES * DMA_ROWS * 2
    inp = rng.standard_normal((B, D)).astype(np.float32)
    w = rng.random((D, 3)).astype(np.float32)
    W_lin = (rng.standard_normal((D, D)) / np.sqrt(D)).astype(np.float32)
    b_lin = (rng.standard_normal(D) * 0.01).astype(np.float32)
    dt = np.ones(1, np.float32)

    actual = kernel(inp=inp, dt=dt, w=w, W_lin=W_lin, b_lin=b_lin)

    a = inp.astype(np.float64)
    c1 = (w[:, 0] * w[:, 2]).astype(np.float64)
    c2 = (w[:, 1] * w[:, 2]).astype(np.float64)
    c1[1] = w[1, 0] * w[1, 0]
    c2[1] = w[1, 1] * w[1, 0]
    ap1 = np.roll(a, -1, 1)
    am2 = np.roll(a, 2, 1)
    am1 = np.roll(a, 1, 1)
    x = (c1 * ap1 - c2 * am2) * am1
    expected = x + a @ W_lin.astype(np.float64).T + b_lin
    err = np.abs(actual - expected).max() / np.abs(expected).max()
    print("scale-relative absmax err:", err)



# revision 2
# speedup vs baseline: 1.1565x; 1.1565x over previous
"""Trainium2 Bass kernel for the BINN convnet problem (transposed bf16 design).

Computation (per row b of inp, all column indices mod D=128):
    x[b, j]  = (c1[j] * a[b, j+1] - c2[j] * a[b, j-2]) * a[b, j-1]
    out      = x + a @ W_lin.T + b_lin
with c1[j] = w[j,0]*outer[j], c2[j] = w[j,1]*outer[j], outer[j] = w[j,2]
except j==1 where outer is w[1,0].

Defining G[j, (j+1)%D] = c1[j], G[j, (j-2)%D] = -c2[j]:
    g = a @ G.T;  x[b, j] = g[b, j] * a[b, j-1];  out = x + a @ W_lin.T + b_lin

Strategy: pure data parallel across 8 NeuronCores (batch split), computing in
FEATURE-TRANSPOSED space with a row-rolled coordinate change so every on-chip
elementwise op is perfectly aligned:

  - The host uploads aT = shard.T as bf16 [128, nrows]: features live on
    partitions, batch on the free dim.  No on-chip transpose is needed, and
    the j-1 stencil shift becomes a partition shift, which a row-roll of the
    CONSTANT matrices absorbs:
        Gs = roll(G, -1, rows); Ws = roll(W_lin, -1, rows); bs = roll(b_lin, -1)
        gs  = Gs @ aT           (gs[j]  = g^T[j+1])
        ms  = Ws @ aT + bs      (ms[j]  = (a@W^T + b)^T[j+1])
        xs  = aT * gs           (aligned elementwise, no wrap)
        outs = xs + ms          (outs[j] = out^T[j+1])
    The host un-rolls the output: out = roll(outs, +1, rows).T.
  - Per 1024-col subtile: 4 bf16 matmuls (N=512 each, stationary Gs^T / Ws^T
    consts) write gs/ms to PSUM fp32; ScalarE evacuates to SBUF bf16, fusing
    the bias add into the ms evacuation (per-partition activation bias).
  - DVE runs the mul and add as bf16 tensor_tensor in 2x_1P mode (16-bit,
    step 1, 4B-aligned) over 2048-col groups.
  - bf16 halves the DMA traffic (16 MiB in + 16 MiB out per core); loads ride
    the SP HWDGE ring (8192-col tiles), stores the ACT ring (4096-col tiles).
"""

import os
import sys

import numpy as np

if os.path.isdir("/opt/trn_rl_repo") and "/opt/trn_rl_repo" not in sys.path:
    sys.path.insert(0, "/opt/trn_rl_repo")

import ml_dtypes

import concourse.mybir as mybir
import concourse.tile as tile
from concourse import bacc
from concourse.bass_utils import run_bass_kernel_spmd

D = 128            # feature dim
N_CORES = 8
SUB = 1024         # cols per PSUM subtile (2 banks per tensor)
GRP = 2048         # cols per DVE group (2 subtiles)
T_LOAD = 8192      # cols per load DMA tile
T_STORE = 4096     # cols per store DMA tile
BF16 = mybir.dt.bfloat16
F32 = mybir.dt.float32
NP_BF16 = ml_dtypes.bfloat16


def build_program(nrows: int):
    """Build the single-core Bass program (SPMD across cores)."""
    assert nrows % T_LOAD == 0
    nload = nrows // T_LOAD
    ngrp = nrows // GRP

    nc = bacc.Bacc("TRN2", debug=False, target_bir_lowering=False)

    at = nc.declare_dram_parameter("at", [D, nrows], BF16, isOutput=False)
    gst = nc.declare_dram_parameter("gst", [D, D], BF16, isOutput=False)
    wst = nc.declare_dram_parameter("wst", [D, D], BF16, isOutput=False)
    bs = nc.declare_dram_parameter("bs", [D, 1], F32, isOutput=False)
    out = nc.declare_dram_parameter("out", [D, nrows], BF16, isOutput=True)

    with tile.TileContext(nc) as tc:
        with (
            tc.tile_pool(name="const", bufs=1) as const_pool,
            tc.tile_pool(name="a_sb", bufs=3) as a_pool,
            tc.tile_pool(name="o_sb", bufs=3) as o_pool,
            tc.tile_pool(name="gs_sb", bufs=2) as gs_pool,
            tc.tile_pool(name="ms_sb", bufs=2) as ms_pool,
            tc.tile_pool(name="xs_sb", bufs=2) as xs_pool,
            tc.tile_pool(name="gs_ps", bufs=2, space="PSUM") as gsps_pool,
            tc.tile_pool(name="ms_ps", bufs=2, space="PSUM") as msps_pool,
        ):
            # --- constants (tiny), on the ACT ring so the SP ring starts
            # with the first data tile ---
            gst_sb = const_pool.tile([D, D], BF16)
            wst_sb = const_pool.tile([D, D], BF16)
            bs_sb = const_pool.tile([D, 1], F32)
            nc.scalar.dma_start(out=gst_sb[:], in_=gst[:, :])
            nc.scalar.dma_start(out=wst_sb[:], in_=wst[:, :])
            nc.scalar.dma_start(out=bs_sb[:], in_=bs[:, :])

            st = {}  # group -> (a_sb, o_sb, goff_in_a, goff_in_o)

            for g in range(ngrp):
                c0 = g * GRP
                # --- load (SP HWDGE ring) ---
                if c0 % T_LOAD == 0:
                    a_sb = a_pool.tile([D, T_LOAD], BF16, tag="a")
                    nc.sync.dma_start(out=a_sb[:], in_=at[:, c0 : c0 + T_LOAD])
                if c0 % T_STORE == 0:
                    o_sb = o_pool.tile([D, T_STORE], BF16, tag="o")
                ga = c0 % T_LOAD
                go = c0 % T_STORE
                st[g] = (a_sb, o_sb, ga, go)

                gs_sb = gs_pool.tile([D, GRP], BF16, tag="gs")
                ms_sb = ms_pool.tile([D, GRP], BF16, tag="ms")
                for s in range(GRP // SUB):
                    soff = ga + s * SUB
                    # --- gs/ms = [Gs|Ws] @ aT subtile (PE, bf16) ---
                    gs_ps = gsps_pool.tile([D, SUB], F32, tag="gsps")
                    ms_ps = msps_pool.tile([D, SUB], F32, tag="msps")
                    for h in range(2):
                        hs = h * 512
                        nc.tensor.matmul(
                            out=gs_ps[:, hs : hs + 512],
                            lhsT=gst_sb[:],
                            rhs=a_sb[:, soff + hs : soff + hs + 512],
                            start=True,
                            stop=True,
                        )
                    for h in range(2):
                        hs = h * 512
                        nc.tensor.matmul(
                            out=ms_ps[:, hs : hs + 512],
                            lhsT=wst_sb[:],
                            rhs=a_sb[:, soff + hs : soff + hs + 512],
                            start=True,
                            stop=True,
                        )
                    # --- evacuate to SBUF bf16 (ScalarE); bias rides the
                    # ms evac as a per-partition activation bias ---
                    eoff = s * SUB
                    nc.scalar.copy(out=gs_sb[:, eoff : eoff + SUB], in_=gs_ps[:])
                    nc.scalar.add(
                        out=ms_sb[:, eoff : eoff + SUB], in_=ms_ps[:], add=bs_sb[:]
                    )

                # --- xs = aT * gs ; outs = xs + ms (DVE, bf16 2x) ---
                xs_sb = xs_pool.tile([D, GRP], BF16, tag="xs")
                nc.vector.tensor_mul(
                    out=xs_sb[:], in0=a_sb[:, ga : ga + GRP], in1=gs_sb[:]
                )
                nc.vector.tensor_add(
                    out=o_sb[:, go : go + GRP], in0=xs_sb[:], in1=ms_sb[:]
                )

                # --- store (ACT HWDGE ring) ---
                if (c0 + GRP) % T_STORE == 0:
                    r0 = (c0 + GRP) - T_STORE
                    nc.scalar.dma_start(
                        out=out[:, r0 : r0 + T_STORE], in_=o_sb[:]
                    )

    nc.compile()
    return nc


def make_consts(w: np.ndarray, W_lin: np.ndarray, b_lin: np.ndarray):
    """Host-side constant preparation (all tiny)."""
    w = np.asarray(w, np.float64)
    c1 = w[:, 0] * w[:, 2]
    c2 = w[:, 1] * w[:, 2]
    # column 1 uses w[1,0] as the outer factor (faithful to source)
    c1[1] = w[1, 0] * w[1, 0]
    c2[1] = w[1, 1] * w[1, 0]

    j = np.arange(D)
    G = np.zeros((D, D), np.float64)
    G[j, (j + 1) % D] += c1
    G[j, (j - 2) % D] -= c2

    Gs = np.roll(G, -1, axis=0)
    Ws = np.roll(np.asarray(W_lin, np.float64), -1, axis=0)
    bsv = np.roll(np.asarray(b_lin, np.float32), -1)

    gst = np.ascontiguousarray(Gs.T).astype(NP_BF16)
    wst = np.ascontiguousarray(Ws.T).astype(NP_BF16)
    bs = np.ascontiguousarray(bsv[:, None].astype(np.float32))
    return {"gst": gst, "wst": wst, "bs": bs}


_PROGRAM_CACHE: dict[int, object] = {}
TRACE = False      # test-only: capture NTFF profile on the next kernel() call
TRACE_DIR = None   # test-only: where to keep NTFF/perfetto artifacts
LAST_RESULT = None  # test-only: BassKernelResults of the last run


def _get_program(nrows: int):
    if nrows not in _PROGRAM_CACHE:
        _PROGRAM_CACHE[nrows] = build_program(nrows)
    return _PROGRAM_CACHE[nrows]


def kernel(**inputs) -> np.ndarray:
    inp = np.asarray(inputs["inp"])
    w = np.asarray(inputs["w"], np.float32)
    W_lin = np.asarray(inputs["W_lin"], np.float32)
    b_lin = np.asarray(inputs["b_lin"], np.float32)

    B = inp.shape[0]
    assert inp.shape[1] == D and B % N_CORES == 0
    nrows = B // N_CORES

    consts = make_consts(w, W_lin, b_lin)
    inp_bf = inp.astype(NP_BF16)

    nc = _get_program(nrows)
    in_maps = []
    for i in range(N_CORES):
        at = np.ascontiguousarray(inp_bf[i * nrows : (i + 1) * nrows, :].T)
        in_maps.append({"at": at, **consts})
    res = run_bass_kernel_spmd(
        nc, in_maps, list(range(N_CORES)), trace=TRACE, tmpdir=TRACE_DIR
    )
    global LAST_RESULT
    LAST_RESULT = res
    outs = [
        np.roll(np.asarray(res.results[i]["out"]), 1, axis=0).T.astype(np.float32)
        for i in range(N_CORES)
    ]
    return np.ascontiguousarray(np.concatenate(outs, axis=0))


if __name__ == "__main__":
    # quick smoke test on random data vs numpy
    rng = np.random.default_rng(0)
    B = N_CORES * T_LOAD
    inp = rng.standard_normal((B, D)).astype(np.float32)
    w = rng.random((D, 3)).astype(np.float32)
    W_lin = (rng.standard_normal((D, D)) / np.sqrt(D)).astype(np.float32)
    b_lin = (rng.standard_normal(D) * 0.01).astype(np.float32)
    dt = np.ones(1, np.float32)

    actual = kernel(inp=inp, dt=dt, w=w, W_lin=W_lin, b_lin=b_lin)

    a = inp.astype(np.float64)
    c1 = (w[:, 0] * w[:, 2]).astype(np.float64)
    c2 = (w[:, 1] * w[:, 2]).astype(np.float64)
    c1[1] = w[1, 0] * w[1, 0]
    c2[1] = w[1, 1] * w[1, 0]
    ap1 = np.roll(a, -1, 1)
    am2 = np.roll(a, 2, 1)
    am1 = np.roll(a, 1, 1)
    x = (c1 * ap1 - c2 * am2) * am1
    expected = x + a @ W_lin.astype(np.float64).T + b_lin
    err = np.abs(actual - expected).max() / np.abs(expected).max()
    print("scale-relative absmax err:", err)
